# revision 10
# baseline (speedup 1.0000x reference)
"""Causal RoPE attention (B=4, T=2048, D=2048, H=16, Dh=128) on 8 trn2 cores.

Sharding (no collectives): core c handles batch b = c//2 and query-block
parity p = c%2.  T is split into 8 blocks of 256 queries; parity p owns
blocks {2j+p : j=0..3} ("slots").  Slot j attends keys [0, (j+1)*512) —
uniform across cores (parity 0 wastes the trailing 256 masked key columns,
keeping the SPMD program shape-identical on every core).  Each core
computes K/V projections for its full batch, Q projection for its 1024
rows, flash-style causal attention, and the output projection for its
rows.  The host reassembles the interleaved row blocks.

All matmuls run as float32r (full-rate fp32 path on the PE array).
"""

import sys

sys.path.insert(0, "/opt/trn_rl_repo")

import numpy as np

D = 2048
T = 2048
H = 16
DH = 128
B = 4
NSLOT = 4          # query slots per core
QW = 256           # queries per slot
QCOLS = NSLOT * QW # 1024 query columns per core
SCALE = 1.0 / np.sqrt(128.0)
MASKV = -1.0e5

_compiled = {}


def _build_nc(debug=False):
    import concourse.bacc as bacc
    from concourse import mybir
    from concourse.tile import TileContext

    F32R = mybir.dt.float32r
    F32 = mybir.dt.float32
    EXP = mybir.ActivationFunctionType.Exp

    nc = bacc.Bacc(trn_type="TRN2")
    dbg = {}
    if debug:
        dbg["KT"] = nc.dram_tensor("dbgKT", [D, T], F32R, kind="ExternalOutput")
        dbg["QT"] = nc.dram_tensor("dbgQT", [D, QCOLS], F32R,
                                   kind="ExternalOutput")
        dbg["V"] = nc.dram_tensor("dbgV", [T, D], F32R, kind="ExternalOutput")
        dbg["ATTN"] = nc.dram_tensor("dbgATTN", [D, QCOLS], F32R,
                                     kind="ExternalOutput")

    xT_d = nc.dram_tensor("xT", [D, T], F32R, kind="ExternalInput")
    xq_d = nc.dram_tensor("xq", [D, QCOLS], F32R, kind="ExternalInput")
    wkbh_d = nc.dram_tensor("wkbh", [H, D, DH], F32R, kind="ExternalInput")
    wqbh_d = nc.dram_tensor("wqbh", [H, D, DH], F32R, kind="ExternalInput")
    wvT_d = nc.dram_tensor("wvT", [D, D], F32R, kind="ExternalInput")
    woT_d = nc.dram_tensor("woT", [D, D], F32R, kind="ExternalInput")
    cosk_d = nc.dram_tensor("cosk", [DH, T], F32, kind="ExternalInput")
    sink_d = nc.dram_tensor("sink", [DH, T], F32, kind="ExternalInput")
    cosq_d = nc.dram_tensor("cosq", [DH, QCOLS], F32, kind="ExternalInput")
    sinq_d = nc.dram_tensor("sinq", [DH, QCOLS], F32, kind="ExternalInput")
    mask_d = nc.dram_tensor("mask", [16, 128, QW], F32, kind="ExternalInput")
    ones_d = nc.dram_tensor("ones", [128, 2], F32R, kind="ExternalInput")
    out_d = nc.dram_tensor("out", [QCOLS, D], F32, kind="ExternalOutput")

    with TileContext(nc) as tc:
        with tc.tile_pool(name="persist", bufs=1) as persist, \
             tc.tile_pool(name="dram", bufs=1, space="DRAM") as dram:
            # attention outputs, transposed per head [dh, q]; resident to P3
            attn_sb = [
                persist.tile([128, QCOLS], F32R, name=f"attn{h}", tag=f"attn{h}")
                for h in range(H)
            ]
            ones_sb = persist.tile([128, 2], F32R, name="ones")
            nc.sync.dma_start(ones_sb[:], ones_d[:])

            KT_dram = dram.tile([D, T], F32R, name="KTd")
            QT_dram = dram.tile([D, QCOLS], F32R, name="QTd")
            V_dram = dram.tile([T, D], F32R, name="Vd")

            # ---------------- P1a: K projection + RoPE -> KT_dram ---------
            # KT[h*128+dh, t] = sum_d wk[h][d, dh] * xT[d, t], then roped.
            with tc.tile_pool(name="wres", bufs=1) as wpool, \
                 tc.tile_pool(name="xcp", bufs=2) as xcp, \
                 tc.tile_pool(name="tabp", bufs=2) as tabp, \
                 tc.tile_pool(name="evp", bufs=2) as evp, \
                 tc.tile_pool(name="pk", bufs=1, space="PSUM") as pkp:
                for hg in range(2):  # head groups of 8
                    wk_sb = []
                    for hh in range(8):
                        h = hg * 8 + hh
                        wt = wpool.tile([128, 16, 128], F32R, tag=f"wk{hh}",
                                        name=f"wk{h}")
                        nc.sync.dma_start(
                            wt[:], wkbh_d[h].rearrange("(d p) c -> p d c", p=128))
                        wk_sb.append(wt)
                    for ch in range(4):  # 512-wide t chunks
                        sl = slice(ch * 512, (ch + 1) * 512)
                        ck = tabp.tile([128, 512], F32, tag="ck")
                        sk = tabp.tile([128, 512], F32, tag="sk")
                        nc.sync.dma_start(ck[:], cosk_d[:, sl])
                        nc.sync.dma_start(sk[:], sink_d[:, sl])
                        # one PSUM bank per head, accumulated over two d-halves
                        pk = [pkp.tile([128, 512], F32, tag=f"pk{hh}",
                                       name=f"pk{hg}_{ch}_{hh}")
                              for hh in range(8)]
                        for dhalf in range(2):
                            xc = []
                            for dd in range(8):
                                d = dhalf * 8 + dd
                                xt = xcp.tile([128, 512], F32R, tag=f"xc{dd}",
                                              name=f"xc{d}_{hg}_{ch}")
                                nc.sync.dma_start(
                                    xt[:], xT_d[d * 128:(d + 1) * 128, sl])
                                xc.append(xt)
                            for hh in range(8):
                                for dd in range(8):
                                    d = dhalf * 8 + dd
                                    nc.tensor.matmul(
                                        pk[hh][:], wk_sb[hh][:, d, :], xc[dd][:],
                                        start=(d == 0), stop=(d == 15))
                        for hh in range(8):
                            h = hg * 8 + hh
                            t1 = evp.tile([128, 512], F32, tag="t1")
                            t2 = evp.tile([128, 512], F32, tag="t2")
                            ko = evp.tile([128, 512], F32R, tag="ko")
                            nc.vector.tensor_mul(t1[:], pk[hh][:], ck[:])
                            nc.vector.tensor_mul(t2[0:64, :], pk[hh][64:128, :],
                                                 sk[0:64, :])
                            nc.vector.tensor_mul(t2[64:128, :], pk[hh][0:64, :],
                                                 sk[64:128, :])
                            nc.vector.tensor_add(ko[:], t1[:], t2[:])
                            nc.sync.dma_start(KT_dram[h * 128:(h + 1) * 128, sl],
                                              ko[:])

            # ---------------- P1b: Q projection + RoPE -> QT_dram ---------
            with tc.tile_pool(name="wqp", bufs=2) as wqp, \
                 tc.tile_pool(name="xqp", bufs=1) as xqp, \
                 tc.tile_pool(name="tabq", bufs=2) as tabq, \
                 tc.tile_pool(name="evq", bufs=3) as evq, \
                 tc.tile_pool(name="pq", bufs=4, space="PSUM") as pqp:
                xq_sb = []
                for d in range(16):
                    xt = xqp.tile([128, QCOLS], F32R, tag=f"xq{d}", name=f"xq{d}")
                    nc.sync.dma_start(xt[:], xq_d[d * 128:(d + 1) * 128, :])
                    xq_sb.append(xt)
                cq = xqp.tile([128, QCOLS], F32, tag="cq", name="cq")
                sq = xqp.tile([128, QCOLS], F32, tag="sq", name="sq")
                nc.sync.dma_start(cq[:], cosq_d[:])
                nc.sync.dma_start(sq[:], sinq_d[:])
                for h in range(H):
                    wt = wqp.tile([128, 16, 128], F32R, tag="wq", name=f"wq{h}")
                    nc.sync.dma_start(
                        wt[:], wqbh_d[h].rearrange("(d p) c -> p d c", p=128))
                    for ch in range(2):  # 512-wide q chunks
                        sl = slice(ch * 512, (ch + 1) * 512)
                        pq = pqp.tile([128, 512], F32, tag="pq")
                        for d in range(16):
                            nc.tensor.matmul(pq[:], wt[:, d, :], xq_sb[d][:, sl],
                                             start=(d == 0), stop=(d == 15))
                        t1 = evq.tile([128, 512], F32, tag="t1")
                        t2 = evq.tile([128, 512], F32, tag="t2")
                        qo = evq.tile([128, 512], F32R, tag="qo")
                        nc.vector.tensor_mul(t1[:], pq[:], cq[:, sl])
                        nc.vector.tensor_mul(t2[0:64, :], pq[64:128, :], sq[0:64, sl])
                        nc.vector.tensor_mul(t2[64:128, :], pq[0:64, :], sq[64:128, sl])
                        nc.vector.tensor_add(qo[:], t1[:], t2[:])
                        nc.sync.dma_start(QT_dram[h * 128:(h + 1) * 128, sl], qo[:])

            # ---------------- P1c: V projection -> V_dram -----------------
            # V[t, o] = sum_d xT[d, t] * wvT[d, o]
            with tc.tile_pool(name="xvp", bufs=2) as xvp, \
                 tc.tile_pool(name="wvp", bufs=2) as wvp, \
                 tc.tile_pool(name="evv", bufs=3) as evv, \
                 tc.tile_pool(name="pv", bufs=4, space="PSUM") as pvp:
                for oc in range(4):  # output column chunks of 512
                    osl = slice(oc * 512, (oc + 1) * 512)
                    wv_sb = []
                    for d in range(16):
                        wt = wvp.tile([128, 512], F32R, tag=f"wv{d}",
                                      name=f"wv{d}_{oc}")
                        nc.sync.dma_start(wt[:], wvT_d[d * 128:(d + 1) * 128, osl])
                        wv_sb.append(wt)
                    for tb in range(16):  # t blocks of 128
                        xv = []
                        for d in range(16):
                            xt = xvp.tile([128, 128], F32R, tag=f"xv{d}",
                                          name=f"xv{d}_{oc}_{tb}")
                            nc.sync.dma_start(
                                xt[:],
                                xT_d[d * 128:(d + 1) * 128, tb * 128:(tb + 1) * 128])
                            xv.append(xt)
                        pv = pvp.tile([128, 512], F32, tag="pv")
                        for d in range(16):
                            nc.tensor.matmul(pv[:], xv[d][:], wv_sb[d][:],
                                             start=(d == 0), stop=(d == 15))
                        vo = evv.tile([128, 512], F32R, tag="vo")
                        nc.scalar.copy(vo[:], pv[:])
                        nc.sync.dma_start(V_dram[tb * 128:(tb + 1) * 128, osl], vo[:])

            # ---------------- P2: causal attention ------------------------
            with tc.tile_pool(name="mskp", bufs=1) as mskp, \
                 tc.tile_pool(name="qtp", bufs=2) as qtp, \
                 tc.tile_pool(name="kvp", bufs=2) as kvp, \
                 tc.tile_pool(name="sep", bufs=3) as sep, \
                 tc.tile_pool(name="nrm", bufs=2) as nrm, \
                 tc.tile_pool(name="pst", bufs=3, space="PSUM") as pst, \
                 tc.tile_pool(name="pau", bufs=2, space="PSUM") as pau, \
                 tc.tile_pool(name="pdn", bufs=2, space="PSUM") as pdn:
                msk_sb = []
                for m in range(16):
                    mt = mskp.tile([128, QW], F32, tag=f"msk{m}", name=f"msk{m}")
                    nc.sync.dma_start(mt[:], mask_d[m])
                    msk_sb.append(mt)
                for h in range(H):
                    qt = qtp.tile([128, QCOLS], F32R, tag="qt", name=f"qth{h}")
                    nc.sync.dma_start(qt[:], QT_dram[h * 128:(h + 1) * 128, :])
                    # slot-outer: exactly one attU group and one denominator
                    # group open at a time, each in its own PSUM bank (a
                    # matmul's start=True clears has_written for the whole
                    # bank, so concurrent groups must not share banks).
                    for j in range(NSLOT):
                        klen = (j + 1) * 512
                        qsl = slice(j * QW, (j + 1) * QW)
                        kc = kvp.tile([128, klen], F32R, tag="kc",
                                      name=f"kc{h}_{j}")
                        nc.sync.dma_start(
                            kc[:], KT_dram[h * 128:(h + 1) * 128, :klen])
                        vc = kvp.tile([128, 4 * (j + 1), 128], F32R, tag="vc",
                                      name=f"vc{h}_{j}")
                        nc.sync.dma_start(
                            vc[:],
                            V_dram[:klen, h * 128:(h + 1) * 128].rearrange(
                                "(b p) d -> p b d", p=128))
                        au = pau.tile([128, QW], F32, tag="au", name=f"au{h}_{j}")
                        dn = pdn.tile([2, QW], F32, tag="dn", name=f"dn{h}_{j}")
                        nkb = 4 * (j + 1)
                        for kb in range(nkb):
                            masked = kb >= nkb - 4
                            st = pst.tile([128, QW], F32, tag="st")
                            nc.tensor.matmul(st[:], kc[:, kb * 128:(kb + 1) * 128],
                                             qt[:, qsl], start=True, stop=True)
                            se = sep.tile([128, QW], F32R, tag="se")
                            if masked:
                                sm = sep.tile([128, QW], F32, tag="sm")
                                nc.vector.tensor_add(
                                    sm[:], st[:],
                                    msk_sb[j * 4 + (kb - (nkb - 4))][:])
                                nc.scalar.activation(se[:], sm[:], EXP,
                                                     scale=SCALE)
                            else:
                                nc.scalar.activation(se[:], st[:], EXP,
                                                     scale=SCALE)
                            nc.tensor.matmul(au[:], vc[:, kb, :], se[:],
                                             start=(kb == 0),
                                             stop=(kb == nkb - 1))
                            nc.tensor.matmul(dn[:], ones_sb[:], se[:],
                                             start=(kb == 0),
                                             stop=(kb == nkb - 1))
                        rec = nrm.tile([1, QW], F32, tag="rec")
                        nc.vector.reciprocal(rec[:], dn[:1, :])
                        rbc = nrm.tile([128, QW], F32, tag="rbc")
                        nc.gpsimd.partition_broadcast(rbc[:], rec[:])
                        nc.vector.tensor_mul(
                            attn_sb[h][:, j * QW:(j + 1) * QW], au[:], rbc[:])

            # ---------------- P3: output projection ------------------------
            # out[r, o] = sum_h sum_dh attn[h][dh, r] * woT[h*128+dh, o]
            with tc.tile_pool(name="wop", bufs=3) as wop, \
                 tc.tile_pool(name="evo", bufs=3) as evo, \
                 tc.tile_pool(name="po", bufs=2, space="PSUM") as pop:
                for oc in range(4):       # out col chunks of 512
                    osl = slice(oc * 512, (oc + 1) * 512)
                    for rg in range(2):   # row groups of 512 rows
                        po = [pop.tile([128, 512], F32, tag=f"po{rb}", name=f"po{oc}_{rg}_{rb}")
                              for rb in range(4)]
                        for h in range(H):
                            wt = wop.tile([128, 512], F32R, tag="wo",
                                          name=f"wo{oc}_{rg}_{h}")
                            nc.sync.dma_start(
                                wt[:], woT_d[h * 128:(h + 1) * 128, osl])
                            for rb in range(4):
                                r = rg * 4 + rb
                                nc.tensor.matmul(
                                    po[rb][:],
                                    attn_sb[h][:, r * 128:(r + 1) * 128], wt[:],
                                    start=(h == 0), stop=(h == H - 1))
                        for rb in range(4):
                            r = rg * 4 + rb
                            oo = evo.tile([128, 512], F32, tag="oo")
                            nc.scalar.copy(oo[:], po[rb][:])
                            nc.sync.dma_start(out_d[r * 128:(r + 1) * 128, osl],
                                              oo[:])

            if debug:
                nc.sync.dma_start(dbg["KT"][:], KT_dram[:])
                nc.sync.dma_start(dbg["QT"][:], QT_dram[:])
                nc.sync.dma_start(dbg["V"][:], V_dram[:])
                for h in range(H):
                    nc.sync.dma_start(dbg["ATTN"][h * 128:(h + 1) * 128, :],
                                      attn_sb[h][:])

    nc.compile()
    return nc


def _host_prep(x, rope_cos, rope_sin, w_q, w_k, w_v, w_o):
    f32 = np.float32
    x = np.ascontiguousarray(x, dtype=f32)
    cosT = np.ascontiguousarray(rope_cos.T, dtype=f32)   # [128, T]
    sinT = np.ascontiguousarray(rope_sin.T, dtype=f32)
    sinTs = sinT.copy()
    sinTs[:64] = -sinTs[:64]

    def byhead(w):
        # w is [out, in]; wT[d, o] = w[o, d]; byhead[h, d, c] = wT[d, h*128+c]
        return np.ascontiguousarray(
            w.T.reshape(D, H, DH).transpose(1, 0, 2), dtype=f32)

    wkbh = byhead(w_k)
    wqbh = byhead(w_q)
    wvT = np.ascontiguousarray(w_v.T, dtype=f32)
    woT = np.ascontiguousarray(w_o.T, dtype=f32)
    ones = np.ones((128, 2), dtype=f32)

    qrows = {}
    masks = {}
    for p in range(2):
        blocks = [2 * j + p for j in range(NSLOT)]
        rows = np.concatenate([np.arange(b * QW, (b + 1) * QW) for b in blocks])
        qrows[p] = rows
        mk = np.empty((16, 128, QW), dtype=f32)
        for j in range(NSLOT):
            qglob = (2 * j + p) * QW + np.arange(QW)[None, :]
            for kb in range(4):
                kglob = (4 * j + kb) * 128 + np.arange(128)[:, None]
                mk[j * 4 + kb] = np.where(qglob >= kglob, 0.0, MASKV)
        masks[p] = mk

    in_maps = []
    for c in range(8):
        b, p = c // 2, c % 2
        xT = np.ascontiguousarray(x[b].T)           # [D, T]
        xq = np.ascontiguousarray(xT[:, qrows[p]])  # [D, 1024]
        in_maps.append({
            "xT": xT,
            "xq": xq,
            "wkbh": wkbh,
            "wqbh": wqbh,
            "wvT": wvT,
            "woT": woT,
            "cosk": cosT,
            "sink": sinTs,
            "cosq": np.ascontiguousarray(cosT[:, qrows[p]]),
            "sinq": np.ascontiguousarray(sinTs[:, qrows[p]]),
            "mask": masks[p],
            "ones": ones,
        })
    return in_maps, qrows


def kernel(x, rope_cos, rope_sin, w_q, w_k, w_v, w_o):
    from concourse.bass_utils import run_bass_kernel_spmd

    if "nc" not in _compiled:
        _compiled["nc"] = _build_nc()
    nc = _compiled["nc"]

    in_maps, qrows = _host_prep(np.asarray(x), np.asarray(rope_cos),
                                np.asarray(rope_sin), np.asarray(w_q),
                                np.asarray(w_k), np.asarray(w_v),
                                np.asarray(w_o))
    res = run_bass_kernel_spmd(nc, in_maps, core_ids=list(range(8)))
    out = np.empty((B, T, D), dtype=np.float32)
    for c in range(8):
        b, p = c // 2, c % 2
        out[b, qrows[p], :] = res.results[c]["out"]
    return out


# revision 12
# speedup vs baseline: 5.6585x; 5.6585x over previous
"""Causal RoPE attention (B=4, T=2048, D=2048, H=16, Dh=128) on 8 trn2 cores.

Sharding (no collectives): core c handles batch b = c//2 and query-block
parity p = c%2.  T is split into 8 blocks of 256 queries; parity p owns
blocks {2j+p : j=0..3} ("slots").  Slot j attends keys [0, (j+1)*512) —
uniform across cores (parity 0 wastes the trailing 256 masked key columns,
keeping the SPMD program shape-identical on every core).  Each core
computes K/V projections for its full batch, Q projection for its 1024
rows, flash-style causal attention, and the output projection for its
rows.  The host reassembles the interleaved row blocks.

All matmuls run as float32r (full-rate fp32 path on the PE array).
"""

import sys

sys.path.insert(0, "/opt/trn_rl_repo")

import numpy as np

D = 2048
T = 2048
H = 16
DH = 128
B = 4
NSLOT = 4          # query slots per core
QW = 256           # queries per slot
QCOLS = NSLOT * QW # 1024 query columns per core
SCALE = 1.0 / np.sqrt(128.0)
MASKV = -1.0e5

_compiled = {}


def _build_nc(debug=False):
    import concourse.bacc as bacc
    from concourse import mybir
    from concourse.tile import TileContext

    F32R = mybir.dt.float32r
    F32 = mybir.dt.float32
    EXP = mybir.ActivationFunctionType.Exp

    nc = bacc.Bacc(trn_type="TRN2")
    dbg = {}
    if debug:
        dbg["KT"] = nc.dram_tensor("dbgKT", [D, T], F32R, kind="ExternalOutput")
        dbg["QT"] = nc.dram_tensor("dbgQT", [D, QCOLS], F32R,
                                   kind="ExternalOutput")
        dbg["V"] = nc.dram_tensor("dbgV", [T, D], F32R, kind="ExternalOutput")
        dbg["ATTN"] = nc.dram_tensor("dbgATTN", [D, QCOLS], F32R,
                                     kind="ExternalOutput")

    xT_d = nc.dram_tensor("xT", [D, T], F32R, kind="ExternalInput")
    xq_d = nc.dram_tensor("xq", [D, QCOLS], F32R, kind="ExternalInput")
    wkbh_d = nc.dram_tensor("wkbh", [H, D, DH], F32R, kind="ExternalInput")
    wqbh_d = nc.dram_tensor("wqbh", [H, D, DH], F32R, kind="ExternalInput")
    wvT_d = nc.dram_tensor("wvT", [D, D], F32R, kind="ExternalInput")
    woT_d = nc.dram_tensor("woT", [D, D], F32R, kind="ExternalInput")
    cosk_d = nc.dram_tensor("cosk", [DH, T], F32, kind="ExternalInput")
    sink_d = nc.dram_tensor("sink", [DH, T], F32, kind="ExternalInput")
    cosq_d = nc.dram_tensor("cosq", [DH, QCOLS], F32, kind="ExternalInput")
    sinq_d = nc.dram_tensor("sinq", [DH, QCOLS], F32, kind="ExternalInput")
    mask_d = nc.dram_tensor("mask", [16, 128, QW], F32, kind="ExternalInput")
    ones_d = nc.dram_tensor("ones", [128, 2], F32R, kind="ExternalInput")
    out_d = nc.dram_tensor("out", [QCOLS, D], F32, kind="ExternalOutput")

    with TileContext(nc) as tc:
        with tc.tile_pool(name="persist", bufs=1) as persist, \
             tc.tile_pool(name="dram", bufs=1, space="DRAM") as dram:
            # attention outputs, transposed per head [dh, q]; resident to P3
            attn_sb = [
                persist.tile([128, QCOLS], F32R, name=f"attn{h}", tag=f"attn{h}")
                for h in range(H)
            ]
            ones_sb = persist.tile([128, 2], F32R, name="ones")
            nc.sync.dma_start(ones_sb[:], ones_d[:])

            KT_dram = dram.tile([D, T], F32R, name="KTd")
            QT_dram = dram.tile([D, QCOLS], F32R, name="QTd")
            V_dram = dram.tile([T, D], F32R, name="Vd")

            # ---------------- P1a: K projection + RoPE -> KT_dram ---------
            # KT[h*128+dh, t] = sum_d wk[h][d, dh] * xT[d, t], then roped.
            with tc.tile_pool(name="wres", bufs=1) as wpool, \
                 tc.tile_pool(name="xcp", bufs=2) as xcp, \
                 tc.tile_pool(name="tabp", bufs=2) as tabp, \
                 tc.tile_pool(name="evp", bufs=2) as evp, \
                 tc.tile_pool(name="pk", bufs=1, space="PSUM") as pkp:
                for hg in range(2):  # head groups of 8
                    wk_sb = []
                    for hh in range(8):
                        h = hg * 8 + hh
                        wt = wpool.tile([128, 16, 128], F32R, tag=f"wk{hh}",
                                        name=f"wk{h}")
                        nc.sync.dma_start(
                            wt[:], wkbh_d[h].rearrange("(d p) c -> p d c", p=128))
                        wk_sb.append(wt)
                    for ch in range(4):  # 512-wide t chunks
                        sl = slice(ch * 512, (ch + 1) * 512)
                        ck = tabp.tile([128, 512], F32, tag="ck")
                        sk = tabp.tile([128, 512], F32, tag="sk")
                        nc.sync.dma_start(ck[:], cosk_d[:, sl])
                        nc.sync.dma_start(sk[:], sink_d[:, sl])
                        # one PSUM bank per head, accumulated over two d-halves
                        pk = [pkp.tile([128, 512], F32, tag=f"pk{hh}",
                                       name=f"pk{hg}_{ch}_{hh}")
                              for hh in range(8)]
                        for dhalf in range(2):
                            xc = []
                            for dd in range(8):
                                d = dhalf * 8 + dd
                                xt = xcp.tile([128, 512], F32R, tag=f"xc{dd}",
                                              name=f"xc{d}_{hg}_{ch}")
                                nc.sync.dma_start(
                                    xt[:], xT_d[d * 128:(d + 1) * 128, sl])
                                xc.append(xt)
                            for hh in range(8):
                                for dd in range(8):
                                    d = dhalf * 8 + dd
                                    nc.tensor.matmul(
                                        pk[hh][:], wk_sb[hh][:, d, :], xc[dd][:],
                                        start=(d == 0), stop=(d == 15))
                        for hh in range(8):
                            h = hg * 8 + hh
                            t1 = evp.tile([128, 512], F32, tag="t1")
                            t2 = evp.tile([128, 512], F32, tag="t2")
                            ko = evp.tile([128, 512], F32R, tag="ko")
                            nc.vector.tensor_mul(t1[:], pk[hh][:], ck[:])
                            nc.vector.tensor_mul(t2[0:64, :], pk[hh][64:128, :],
                                                 sk[0:64, :])
                            nc.vector.tensor_mul(t2[64:128, :], pk[hh][0:64, :],
                                                 sk[64:128, :])
                            nc.vector.tensor_add(ko[:], t1[:], t2[:])
                            nc.sync.dma_start(KT_dram[h * 128:(h + 1) * 128, sl],
                                              ko[:])

            # ---------------- P1b: Q projection + RoPE -> QT_dram ---------
            with tc.tile_pool(name="wqp", bufs=2) as wqp, \
                 tc.tile_pool(name="xqp", bufs=1) as xqp, \
                 tc.tile_pool(name="tabq", bufs=2) as tabq, \
                 tc.tile_pool(name="evq", bufs=3) as evq, \
                 tc.tile_pool(name="pq", bufs=4, space="PSUM") as pqp:
                xq_sb = []
                for d in range(16):
                    xt = xqp.tile([128, QCOLS], F32R, tag=f"xq{d}", name=f"xq{d}")
                    nc.sync.dma_start(xt[:], xq_d[d * 128:(d + 1) * 128, :])
                    xq_sb.append(xt)
                cq = xqp.tile([128, QCOLS], F32, tag="cq", name="cq")
                sq = xqp.tile([128, QCOLS], F32, tag="sq", name="sq")
                nc.sync.dma_start(cq[:], cosq_d[:])
                nc.sync.dma_start(sq[:], sinq_d[:])
                for h in range(H):
                    wt = wqp.tile([128, 16, 128], F32R, tag="wq", name=f"wq{h}")
                    nc.sync.dma_start(
                        wt[:], wqbh_d[h].rearrange("(d p) c -> p d c", p=128))
                    for ch in range(2):  # 512-wide q chunks
                        sl = slice(ch * 512, (ch + 1) * 512)
                        pq = pqp.tile([128, 512], F32, tag="pq")
                        for d in range(16):
                            nc.tensor.matmul(pq[:], wt[:, d, :], xq_sb[d][:, sl],
                                             start=(d == 0), stop=(d == 15))
                        t1 = evq.tile([128, 512], F32, tag="t1")
                        t2 = evq.tile([128, 512], F32, tag="t2")
                        qo = evq.tile([128, 512], F32R, tag="qo")
                        nc.vector.tensor_mul(t1[:], pq[:], cq[:, sl])
                        nc.vector.tensor_mul(t2[0:64, :], pq[64:128, :], sq[0:64, sl])
                        nc.vector.tensor_mul(t2[64:128, :], pq[0:64, :], sq[64:128, sl])
                        nc.vector.tensor_add(qo[:], t1[:], t2[:])
                        nc.sync.dma_start(QT_dram[h * 128:(h + 1) * 128, sl], qo[:])

            # ---------------- P1c: V projection -> V_dram -----------------
            # V[t, o] = sum_d xT[d, t] * wvT[d, o]
            with tc.tile_pool(name="xvp", bufs=2) as xvp, \
                 tc.tile_pool(name="wvp", bufs=2) as wvp, \
                 tc.tile_pool(name="evv", bufs=3) as evv, \
                 tc.tile_pool(name="pv", bufs=4, space="PSUM") as pvp:
                for oc in range(4):  # output column chunks of 512
                    osl = slice(oc * 512, (oc + 1) * 512)
                    wv_sb = []
                    for d in range(16):
                        wt = wvp.tile([128, 512], F32R, tag=f"wv{d}",
                                      name=f"wv{d}_{oc}")
                        nc.sync.dma_start(wt[:], wvT_d[d * 128:(d + 1) * 128, osl])
                        wv_sb.append(wt)
                    for tb in range(16):  # t blocks of 128
                        xv = []
                        for d in range(16):
                            xt = xvp.tile([128, 128], F32R, tag=f"xv{d}",
                                          name=f"xv{d}_{oc}_{tb}")
                            nc.sync.dma_start(
                                xt[:],
                                xT_d[d * 128:(d + 1) * 128, tb * 128:(tb + 1) * 128])
                            xv.append(xt)
                        pv = pvp.tile([128, 512], F32, tag="pv")
                        for d in range(16):
                            nc.tensor.matmul(pv[:], xv[d][:], wv_sb[d][:],
                                             start=(d == 0), stop=(d == 15))
                        vo = evv.tile([128, 512], F32R, tag="vo")
                        nc.scalar.copy(vo[:], pv[:])
                        nc.sync.dma_start(V_dram[tb * 128:(tb + 1) * 128, osl], vo[:])

            # ---------------- P2: causal attention ------------------------
            with tc.tile_pool(name="mskp", bufs=1) as mskp, \
                 tc.tile_pool(name="qtp", bufs=2) as qtp, \
                 tc.tile_pool(name="kvp", bufs=2) as kvp, \
                 tc.tile_pool(name="sep", bufs=3) as sep, \
                 tc.tile_pool(name="nrm", bufs=2) as nrm, \
                 tc.tile_pool(name="pst", bufs=3, space="PSUM") as pst, \
                 tc.tile_pool(name="pau", bufs=1, space="PSUM") as pau, \
                 tc.tile_pool(name="pdn", bufs=1, space="PSUM") as pdn:
                msk_sb = []
                for m in range(16):
                    mt = mskp.tile([128, QW], F32, tag=f"msk{m}", name=f"msk{m}")
                    nc.sync.dma_start(mt[:], mask_d[m])
                    msk_sb.append(mt)
                for h in range(H):
                    qt = qtp.tile([128, QCOLS], F32R, tag="qt", name=f"qth{h}")
                    nc.sync.dma_start(qt[:], QT_dram[h * 128:(h + 1) * 128, :])
                    # Slot pairs with chunk-outer k loop: each open
                    # accumulation group (attU / denominator per slot) has its
                    # own PSUM bank — a matmul's start=True clears has_written
                    # for its whole bank, so concurrent groups must not share.
                    for a in (0, 2):  # slot pairs (0,1) and (2,3)
                        au = {j: pau.tile([128, QW], F32, tag=f"au{j - a}",
                                          name=f"au{h}_{j}")
                              for j in (a, a + 1)}
                        dn = {j: pdn.tile([2, QW], F32, tag=f"dn{j - a}",
                                          name=f"dn{h}_{j}")
                              for j in (a, a + 1)}
                        for c in range(a + 2):  # key chunks of 512
                            kc = kvp.tile([128, 512], F32R, tag="kc",
                                          name=f"kc{h}_{a}_{c}")
                            nc.sync.dma_start(
                                kc[:], KT_dram[h * 128:(h + 1) * 128,
                                               c * 512:(c + 1) * 512])
                            vc = kvp.tile([128, 4, 128], F32R, tag="vc",
                                          name=f"vc{h}_{a}_{c}")
                            nc.sync.dma_start(
                                vc[:],
                                V_dram[c * 512:(c + 1) * 512,
                                       h * 128:(h + 1) * 128].rearrange(
                                           "(b p) d -> p b d", p=128))
                            for j in (a, a + 1):
                                if c > j:
                                    continue
                                qsl = slice(j * QW, (j + 1) * QW)
                                first = (c == 0)
                                last = (c == j)
                                for kb in range(4):
                                    st = pst.tile([128, QW], F32, tag="st")
                                    nc.tensor.matmul(
                                        st[:], kc[:, kb * 128:(kb + 1) * 128],
                                        qt[:, qsl], start=True, stop=True)
                                    se = sep.tile([128, QW], F32R, tag="se")
                                    if last:
                                        sm = sep.tile([128, QW], F32, tag="sm")
                                        nc.vector.tensor_add(
                                            sm[:], st[:], msk_sb[j * 4 + kb][:])
                                        nc.scalar.activation(se[:], sm[:], EXP,
                                                             scale=SCALE)
                                    else:
                                        nc.scalar.activation(se[:], st[:], EXP,
                                                             scale=SCALE)
                                    nc.tensor.matmul(au[j][:], vc[:, kb, :],
                                                     se[:],
                                                     start=(first and kb == 0),
                                                     stop=(last and kb == 3))
                                    nc.tensor.matmul(dn[j][:], ones_sb[:],
                                                     se[:],
                                                     start=(first and kb == 0),
                                                     stop=(last and kb == 3))
                        for j in (a, a + 1):
                            rec = nrm.tile([1, QW], F32, tag="rec")
                            nc.vector.reciprocal(rec[:], dn[j][:1, :])
                            rbc = nrm.tile([128, QW], F32, tag="rbc")
                            nc.gpsimd.partition_broadcast(rbc[:], rec[:])
                            nc.vector.tensor_mul(
                                attn_sb[h][:, j * QW:(j + 1) * QW], au[j][:],
                                rbc[:])

            # ---------------- P3: output projection ------------------------
            # out[r, o] = sum_h sum_dh attn[h][dh, r] * woT[h*128+dh, o]
            with tc.tile_pool(name="wop", bufs=3) as wop, \
                 tc.tile_pool(name="evo", bufs=3) as evo, \
                 tc.tile_pool(name="po", bufs=2, space="PSUM") as pop:
                for oc in range(4):       # out col chunks of 512
                    osl = slice(oc * 512, (oc + 1) * 512)
                    for rg in range(2):   # row groups of 512 rows
                        po = [pop.tile([128, 512], F32, tag=f"po{rb}", name=f"po{oc}_{rg}_{rb}")
                              for rb in range(4)]
                        for h in range(H):
                            wt = wop.tile([128, 512], F32R, tag="wo",
                                          name=f"wo{oc}_{rg}_{h}")
                            nc.sync.dma_start(
                                wt[:], woT_d[h * 128:(h + 1) * 128, osl])
                            for rb in range(4):
                                r = rg * 4 + rb
                                nc.tensor.matmul(
                                    po[rb][:],
                                    attn_sb[h][:, r * 128:(r + 1) * 128], wt[:],
                                    start=(h == 0), stop=(h == H - 1))
                        for rb in range(4):
                            r = rg * 4 + rb
                            oo = evo.tile([128, 512], F32, tag="oo")
                            nc.scalar.copy(oo[:], po[rb][:])
                            nc.sync.dma_start(out_d[r * 128:(r + 1) * 128, osl],
                                              oo[:])

            if debug:
                nc.sync.dma_start(dbg["KT"][:], KT_dram[:])
                nc.sync.dma_start(dbg["QT"][:], QT_dram[:])
                nc.sync.dma_start(dbg["V"][:], V_dram[:])
                for h in range(H):
                    nc.sync.dma_start(dbg["ATTN"][h * 128:(h + 1) * 128, :],
                                      attn_sb[h][:])

    nc.compile()
    return nc


def _host_prep(x, rope_cos, rope_sin, w_q, w_k, w_v, w_o):
    f32 = np.float32
    x = np.ascontiguousarray(x, dtype=f32)
    cosT = np.ascontiguousarray(rope_cos.T, dtype=f32)   # [128, T]
    sinT = np.ascontiguousarray(rope_sin.T, dtype=f32)
    sinTs = sinT.copy()
    sinTs[:64] = -sinTs[:64]

    def byhead(w):
        # w is [out, in]; wT[d, o] = w[o, d]; byhead[h, d, c] = wT[d, h*128+c]
        return np.ascontiguousarray(
            w.T.reshape(D, H, DH).transpose(1, 0, 2), dtype=f32)

    wkbh = byhead(w_k)
    wqbh = byhead(w_q)
    wvT = np.ascontiguousarray(w_v.T, dtype=f32)
    woT = np.ascontiguousarray(w_o.T, dtype=f32)
    ones = np.ones((128, 2), dtype=f32)

    qrows = {}
    masks = {}
    for p in range(2):
        blocks = [2 * j + p for j in range(NSLOT)]
        rows = np.concatenate([np.arange(b * QW, (b + 1) * QW) for b in blocks])
        qrows[p] = rows
        mk = np.empty((16, 128, QW), dtype=f32)
        for j in range(NSLOT):
            qglob = (2 * j + p) * QW + np.arange(QW)[None, :]
            for kb in range(4):
                kglob = (4 * j + kb) * 128 + np.arange(128)[:, None]
                mk[j * 4 + kb] = np.where(qglob >= kglob, 0.0, MASKV)
        masks[p] = mk

    in_maps = []
    for c in range(8):
        b, p = c // 2, c % 2
        xT = np.ascontiguousarray(x[b].T)           # [D, T]
        xq = np.ascontiguousarray(xT[:, qrows[p]])  # [D, 1024]
        in_maps.append({
            "xT": xT,
            "xq": xq,
            "wkbh": wkbh,
            "wqbh": wqbh,
            "wvT": wvT,
            "woT": woT,
            "cosk": cosT,
            "sink": sinTs,
            "cosq": np.ascontiguousarray(cosT[:, qrows[p]]),
            "sinq": np.ascontiguousarray(sinTs[:, qrows[p]]),
            "mask": masks[p],
            "ones": ones,
        })
    return in_maps, qrows


def kernel(x, rope_cos, rope_sin, w_q, w_k, w_v, w_o):
    from concourse.bass_utils import run_bass_kernel_spmd

    if "nc" not in _compiled:
        _compiled["nc"] = _build_nc()
    nc = _compiled["nc"]

    in_maps, qrows = _host_prep(np.asarray(x), np.asarray(rope_cos),
                                np.asarray(rope_sin), np.asarray(w_q),
                                np.asarray(w_k), np.asarray(w_v),
                                np.asarray(w_o))
    res = run_bass_kernel_spmd(nc, in_maps, core_ids=list(range(8)))
    out = np.empty((B, T, D), dtype=np.float32)
    for c in range(8):
        b, p = c // 2, c % 2
        out[b, qrows[p], :] = res.results[c]["out"]
    return out
